# revision 22
# baseline (speedup 1.0000x reference)
"""Trainium2 Bass kernel for DriverNet: 2-layer LSTM cell (single step, zero
initial state) + linear head over B=1M rows, data-parallel on 8 NeuronCores.

v3 design:
- x converted to bf16 host-side (halves HBM read traffic); 22-feature slots
  (21 features + a ones slot memset once into persistent ping-pong tiles)
  so the bias rides the matmul as a weight row.
- PE transposes [128, chunk*22] -> PSUM, DVE evacuates a full bank at a time
  (bf16 2x mode), giving feature-major lhsT tiles for block-diagonal matmuls:
  L0 chunk=4 blocks (K=88, N=60), L1 chunk=16 blocks (K=96, N=240).
- all sigmoids become tanh via sig(z) = (tanh(z/2)+1)/2: the 1/2 folds into
  weights, (t+1)*u maps to one scalar_tensor_tensor DVE op, doubled hidden
  states fold into the next layer's weights. One Tanh per gate matrix
  (3-bank strided PSUM read = the evacuation).
- final linear: t = h2' * (W_lin/2), tensor_reduce(X), Tanh + bias b_lin.
"""

import os
import numpy as np
import ml_dtypes

B = 1 << 20
IN_DIM, HID, OUT_DIM = 21, 5, 1
NCORES = 8
BC = B // NCORES          # 131072 rows per core
NBLK = BC // 128          # 1024 blocks per core
SUPERS = [64] * 16
NBMAX = max(SUPERS)
L0C = 4                   # L0 blocks per chunk
L1C = 16                  # L1 blocks per chunk
L0_PER_BANK = 8           # 8*60 = 480 <= 512 fp32
L1_PER_BANK = 2
YGRP = 4                  # supertiles per y-store group           # 2*240 = 480

_CACHE = {}
LAST_RESULTS = None


def _build_program(reps=1):
    import contextlib
    import concourse.bacc as bacc
    import concourse.tile as tile
    import concourse.mybir as mybir

    AF = mybir.ActivationFunctionType
    ALU = mybir.AluOpType
    BF16 = mybir.dt.bfloat16
    F32 = mybir.dt.float32
    nc = bacc.Bacc("TRN2", target_bir_lowering=False, debug=False, num_devices=NCORES)

    x_d = nc.declare_dram_parameter("xb", [BC, 22], BF16, isOutput=False)
    w0_d = nc.declare_dram_parameter("w0blk", [L0C * 22, L0C * 15], BF16, isOutput=False)
    w1_d = nc.declare_dram_parameter("w1blk", [L1C * 6, L1C * 15], BF16, isOutput=False)
    wr_d = nc.declare_dram_parameter("wrep", [128, NBMAX * HID], BF16, isOutput=False)
    bl_d = nc.declare_dram_parameter("blin", [128, 1], F32, isOutput=False)
    id_d = nc.declare_dram_parameter("ident", [128, 128], BF16, isOutput=False)
    y_d = nc.declare_dram_parameter("y", [BC, 1], F32, isOutput=True)

    env = lambda k, d: int(os.environ.get(k, d))
    with tile.TileContext(nc) as tc:
        with (
            tc.tile_pool(name="const", bufs=1) as constp,
            tc.tile_pool(name="xin", bufs=env("XIN_BUFS", 3)) as xinp,
            tc.tile_pool(name="xt_ps", bufs=env("XTPS_BUFS", 1), space="PSUM") as xtpsp,
            tc.tile_pool(name="xt_sb", bufs=env("XTSB_BUFS", 3)) as xtsbp,
            tc.tile_pool(name="g0_ps", bufs=env("G0_BUFS", 1), space="PSUM") as g0psp,
            tc.tile_pool(name="h1t_ps", bufs=env("H1TPS_BUFS", 1), space="PSUM") as h1tpsp,
            tc.tile_pool(name="h1t_sb", bufs=env("H1TSB_BUFS", 3)) as h1tsbp,
            tc.tile_pool(name="g1_ps", bufs=env("G1_BUFS", 1), space="PSUM") as g1psp,
            tc.tile_pool(name="acts", bufs=env("ACTS_BUFS", 3)) as actsp,
            tc.tile_pool(name="yout", bufs=env("YOUT_BUFS", 2)) as youtp,
        ):
            w0_sb = constp.tile([L0C * 22, L0C * 15], BF16)
            nc.sync.dma_start(w0_sb[:], w0_d[:])
            w1_sb = constp.tile([L1C * 6, L1C * 15], BF16)
            nc.sync.dma_start(w1_sb[:], w1_d[:])
            wr_sb = constp.tile([128, NBMAX * HID], BF16)
            nc.sync.dma_start(wr_sb[:], wr_d[:])
            bl_sb = constp.tile([128, 1], F32)
            nc.sync.dma_start(bl_sb[:], bl_d[:])
            id_sb = constp.tile([128, 128], BF16)
            nc.sync.dma_start(id_sb[:], id_d[:])

            # persistent ping-pong h1 tiles; ones slots memset once
            h1_tiles = []
            for pp in range(2):
                ht = constp.tile([128, NBMAX * 6], BF16, tag=f"h1tile{pp}")
                nc.vector.memset(
                    ht[:].rearrange("p (r f) -> p r f", f=6)[:, :, 5:6], 1.0
                )
                h1_tiles.append(ht)

            if reps > 1:
                rep_ctx = tc.For_i(0, reps, 1, hint_engines=tuple(nc.engines))
            else:
                rep_ctx = contextlib.nullcontext()
            def emit_l0(si, nb, s0):
                S = nb * 128
                n0ch = nb // L0C
                n1ch = nb // L1C
                g0b = n0ch // L0_PER_BANK
                g1b = n1ch // L1_PER_BANK

                # ---- load x shard (bf16, contiguous; ones col from host)
                # split per g0-bank-group so transposes can start early
                x_tile = xinp.tile([128, nb * 22], BF16, tag="xin")
                xs_v = x_d[s0 : s0 + S, :].rearrange("(p r) f -> p (r f)", p=128)
                grp = L0_PER_BANK * L0C * 22
                for bl in range(g0b):
                    nc.gpsimd.dma_start(
                        out=x_tile[:, bl * grp : (bl + 1) * grp],
                        in_=xs_v[:, bl * grp : (bl + 1) * grp],
                    )

                # ---- L0: PE transposes (2-bank staging) + one DVE evac + matmuls
                g0_ps = g0psp.tile([128, g0b * 512], F32, tag="g0")
                xt_ps = xtpsp.tile([L0C * 22, n0ch * 128], BF16, tag="xtps")
                for c in range(n0ch):
                    nc.tensor.transpose(
                        xt_ps[:, c * 128 : (c + 1) * 128],
                        x_tile[:, c * L0C * 22 : (c + 1) * L0C * 22],
                        id_sb[:],
                    )
                xt_sb = xtsbp.tile([L0C * 22, n0ch * 128], BF16, tag="xtsb")
                nc.vector.tensor_copy(xt_sb[:], xt_ps[:])
                for c in range(n0ch):
                    off = (c // L0_PER_BANK) * 512 + (c % L0_PER_BANK) * 60
                    nc.tensor.matmul(
                        g0_ps[:, off : off + 60],
                        xt_sb[:, c * 128 : (c + 1) * 128],
                        w0_sb[:],
                        start=True,
                        stop=True,
                    )

                # ---- L0 elementwise: one big tanh, STT muls
                g0v = (
                    g0_ps[:]
                    .rearrange("p (b x) -> p b x", x=512)[:, :, : L0_PER_BANK * 60]
                    .rearrange("p b (c n) -> p b c n", n=60)
                )
                sio0 = actsp.tile([128, n0ch * 40], BF16, tag="sio0")
                nc.scalar.activation(
                    sio0[:].rearrange("p (b c n) -> p b c n", n=40, c=L0_PER_BANK),
                    g0v[:, :, :, 0:40],
                    AF.Sigmoid,
                )
                tg0 = actsp.tile([128, nb * HID], BF16, tag="tg0")
                nc.scalar.activation(
                    tg0[:].rearrange("p (b c n) -> p b c n", n=20, c=L0_PER_BANK),
                    g0v[:, :, :, 40:60],
                    AF.Tanh,
                )
                sio0v = sio0[:].rearrange("p (c n) -> p c n", n=40)
                c1 = actsp.tile([128, nb * HID], BF16, tag="c1")
                nc.vector.tensor_mul(
                    c1[:].rearrange("p (c n) -> p c n", n=20),
                    sio0v[:, :, 0:20],
                    tg0[:].rearrange("p (c n) -> p c n", n=20),
                )
                tc1 = actsp.tile([128, nb * HID], BF16, tag="tc1")
                nc.scalar.activation(tc1[:], c1[:], AF.Tanh)
                h1 = h1_tiles[si % 2]
                nc.vector.tensor_mul(
                    h1[:].rearrange("p (c d f) -> p c d f", d=L0C, f=6)[:, :n0ch, :, 0:5],
                    sio0[:].rearrange("p (c g d f) -> p c g d f", g=2, d=L0C, f=5)[:, :, 1],
                    tc1[:].rearrange("p (c d f) -> p c d f", d=L0C, f=5),
                )

                return dict(si=si, nb=nb, s0=s0, h1=h1)

            def emit_l1fin(ctx):
                si, nb, s0, h1 = ctx["si"], ctx["nb"], ctx["s0"], ctx["h1"]
                S = nb * 128
                n1ch = nb // L1C
                g1b = n1ch // L1_PER_BANK
                # ---- L1: PE transposes + DVE evac + matmuls
                g1_ps = g1psp.tile([128, g1b * 512], F32, tag="g1")
                h1t_ps = h1tpsp.tile([L1C * 6, n1ch * 128], BF16, tag="h1tps")
                for c in range(n1ch):
                    nc.tensor.transpose(
                        h1t_ps[:, c * 128 : (c + 1) * 128],
                        h1[:, c * L1C * 6 : (c + 1) * L1C * 6],
                        id_sb[:],
                    )
                h1t_sb = h1tsbp.tile([L1C * 6, n1ch * 128], BF16, tag="h1tsb")
                nc.vector.tensor_copy(h1t_sb[:], h1t_ps[:])
                for c in range(n1ch):
                    off = (c // L1_PER_BANK) * 512 + (c % L1_PER_BANK) * 240
                    nc.tensor.matmul(
                        g1_ps[:, off : off + 240],
                        h1t_sb[:, c * 128 : (c + 1) * 128],
                        w1_sb[:],
                        start=True,
                        stop=True,
                    )

                # ---- L1 elementwise
                g1v = (
                    g1_ps[:]
                    .rearrange("p (b x) -> p b x", x=512)[:, :, : L1_PER_BANK * 240]
                    .rearrange("p b (c n) -> p b c n", n=240)
                )
                sio1 = actsp.tile([128, n1ch * 160], BF16, tag="sio1")
                nc.scalar.activation(
                    sio1[:].rearrange("p (b c n) -> p b c n", n=160, c=L1_PER_BANK),
                    g1v[:, :, :, 0:160],
                    AF.Sigmoid,
                )
                tg1 = actsp.tile([128, nb * HID], BF16, tag="tg1")
                nc.scalar.activation(
                    tg1[:].rearrange("p (b c n) -> p b c n", n=80, c=L1_PER_BANK),
                    g1v[:, :, :, 160:240],
                    AF.Tanh,
                )
                sio1v = sio1[:].rearrange("p (c n) -> p c n", n=160)
                c2 = actsp.tile([128, nb * HID], BF16, tag="c2")
                nc.vector.tensor_mul(
                    c2[:].rearrange("p (c n) -> p c n", n=80),
                    sio1v[:, :, 0:80],
                    tg1[:].rearrange("p (c n) -> p c n", n=80),
                )
                tc2 = actsp.tile([128, nb * HID], BF16, tag="tc2")
                nc.scalar.activation(tc2[:], c2[:], AF.Tanh)
                vp = actsp.tile([128, nb * HID], BF16, tag="vp")
                nc.vector.tensor_mul(
                    vp[:].rearrange("p (c n) -> p c n", n=80),
                    sio1v[:, :, 80:160],
                    tc2[:].rearrange("p (c n) -> p c n", n=80),
                )
                t = actsp.tile([128, nb * HID], BF16, tag="t")
                nc.vector.tensor_mul(t[:], vp[:], wr_sb[:, : nb * HID])

                # ---- final reduce + tanh(+bias) + store
                ypre = actsp.tile([128, nb], F32, tag="ypre")
                nc.vector.tensor_reduce(
                    ypre[:].rearrange("p (r o) -> p r o", o=1),
                    t[:].rearrange("p (r f) -> p r f", f=HID),
                    mybir.AxisListType.X,
                    ALU.add,
                )
                y_tile = youtp.tile([128, nb], F32, tag="y")
                nc.scalar.activation(y_tile[:], ypre[:], AF.Tanh, bias=bl_sb[:, 0:1])
                nc.sync.dma_start(
                    out=y_d[s0 : s0 + S, 0:1].rearrange("(p r) o -> p (r o)", p=128),
                    in_=y_tile[:],
                )

            with rep_ctx:
                s0 = 0
                pend = None
                for si, nb in enumerate(SUPERS):
                    ctx = emit_l0(si, nb, s0)
                    if pend is not None:
                        emit_l1fin(pend)
                    pend = ctx
                    s0 += nb * 128
                emit_l1fin(pend)

    nc.compile()
    return nc


def _build_inputs(x, W_ih0, W_hh0, b_ih0, b_hh0, W_ih1, W_hh1, b_ih1, b_hh1, W_lin, b_lin):
    bf16 = ml_dtypes.bfloat16
    b0 = (np.asarray(b_ih0) + np.asarray(b_hh0)).astype(np.float32)
    b1 = (np.asarray(b_ih1) + np.asarray(b_hh1)).astype(np.float32)
    W0 = np.asarray(W_ih0, np.float32)
    W1 = np.asarray(W_ih1, np.float32)
    sel = {"i": range(0, 5), "g": range(10, 15), "o": range(15, 20)}
    gscale = {"i": 1.0, "o": 1.0, "g": 1.0}

    def blockdiag(W, b, chunk, slot, wscale):
        # rows: slot*dr + k  (k < kin: weights*gscale*wscale, k == kin: bias*gscale)
        kin = W.shape[1]
        out = np.zeros((chunk * slot, chunk * 15), np.float32)
        for dr in range(chunk):
            for grp, key in enumerate(("i", "o", "g")):
                gs = gscale[key]
                for kk, gr in enumerate(sel[key]):
                    col = grp * (chunk * 5) + dr * 5 + kk
                    r0 = dr * slot
                    out[r0 : r0 + kin, col] = W[gr, :] * gs * wscale
                    out[r0 + kin, col] = b[gr] * gs
        return out.astype(bf16)

    w0blk = blockdiag(W0, b0, L0C, 22, 1.0)
    w1blk = blockdiag(W1, b1, L1C, 6, 1.0)
    wrep = (
        np.tile(np.asarray(W_lin, np.float32)[0], NBMAX * 128)
        .reshape(128, NBMAX * HID)
        .astype(bf16)
    )
    blin = np.full((128, 1), float(np.asarray(b_lin)[0]), np.float32)
    ident = np.eye(128, dtype=bf16)

    xb = np.empty((B, 22), bf16)
    xb[:, :21] = np.asarray(x, np.float32).astype(bf16)
    xb[:, 21] = bf16(1.0)

    in_maps = []
    for c in range(NCORES):
        in_maps.append(
            {
                "xb": xb[c * BC : (c + 1) * BC],
                "w0blk": w0blk,
                "w1blk": w1blk,
                "wrep": wrep,
                "blin": blin,
                "ident": ident,
            }
        )
    return in_maps


def _reference_numpy(x, h0, c0, W_ih0, W_hh0, b_ih0, b_hh0, W_ih1, W_hh1, b_ih1, b_hh1, W_lin, b_lin):
    # general fallback (never taken for the spec'd zero-state inputs)
    def cell(x_, h, c, Wi, Wh, bi, bh):
        g = x_ @ Wi.T + h @ Wh.T + (bi + bh)
        i, f, gg, o = np.split(g, 4, axis=-1)
        sig = lambda z: 1.0 / (1.0 + np.exp(-z))
        cn = sig(f) * c + sig(i) * np.tanh(gg)
        return sig(o) * np.tanh(cn), cn

    h1, _ = cell(x, h0[0], c0[0], W_ih0, W_hh0, b_ih0, b_hh0)
    h2, _ = cell(h1, h0[1], c0[1], W_ih1, W_hh1, b_ih1, b_hh1)
    return np.tanh(h2 @ W_lin.T + b_lin).astype(np.float32)


def kernel(x, h0, c0, W_ih0, W_hh0, b_ih0, b_hh0, W_ih1, W_hh1, b_ih1, b_hh1, W_lin, b_lin):
    global LAST_RESULTS
    args = dict(
        x=np.asarray(x), h0=np.asarray(h0), c0=np.asarray(c0),
        W_ih0=np.asarray(W_ih0), W_hh0=np.asarray(W_hh0),
        b_ih0=np.asarray(b_ih0), b_hh0=np.asarray(b_hh0),
        W_ih1=np.asarray(W_ih1), W_hh1=np.asarray(W_hh1),
        b_ih1=np.asarray(b_ih1), b_hh1=np.asarray(b_hh1),
        W_lin=np.asarray(W_lin), b_lin=np.asarray(b_lin),
    )
    if np.any(args["h0"]) or np.any(args["c0"]):
        return _reference_numpy(**args)

    from concourse.bass_utils import run_bass_kernel_spmd

    if "nc" not in _CACHE:
        _CACHE["nc"] = _build_program()
    nc = _CACHE["nc"]

    in_maps = _build_inputs(
        args["x"], args["W_ih0"], args["W_hh0"], args["b_ih0"], args["b_hh0"],
        args["W_ih1"], args["W_hh1"], args["b_ih1"], args["b_hh1"],
        args["W_lin"], args["b_lin"],
    )
    trace = bool(int(os.environ.get("TRN_TRACE", "0")))
    res = run_bass_kernel_spmd(nc, in_maps, list(range(NCORES)), trace=trace)
    LAST_RESULTS = res
    return np.concatenate([res.results[i]["y"] for i in range(NCORES)], axis=0)


# revision 47
# speedup vs baseline: 1.1348x; 1.1348x over previous
"""Trainium2 Bass kernel for DriverNet: 2-layer LSTM cell (single step, zero
initial state) + linear head over B=1M rows, data-parallel on 8 NeuronCores.

Measured: ~78 us HW exec for the full batch (8 cores; interleaved For_i
repeat-delta method incl ~2-3us loop overhead; cost model: 75.8us), rel
err 0.52%.

Design (per core: 131072 rows = 1024 blocks of 128, supertiles of 64 blocks):
- x is converted to bf16 host-side and a ones column appended ([BC, 22]), so
  the HBM read halves and the gate bias rides the matmul as a weight row.
- all bf16 constants (identity, block-diag weights, tiled W_lin) are packed
  into ONE dram tensor -> single prologue DMA; the sigmoid/tanh ACT table
  load (~2.7us) is pre-triggered by dummy activations so it overlaps the
  first x-load. x-loads go through HWDGE (nc.sync) - SWDGE descriptor
  emission costs ~1.6us of Q7 time per DMA and nearly saturates Pool.SEQ.
- batch lives on SBUF partitions for all elementwise work. Feature-major
  lhsT tiles come from PE transposes [128, 4*22] -> PSUM bf16 (staged 16
  chunks = 2 banks), evacuated by one DVE copy (bf16 2x mode).
- block-diagonal weights (host-built) evaluate several row-blocks per
  matmul: L0 chunk=4 blocks (K=88, N=60, 8 chunks/bank), L1 chunk=16 blocks
  (K=96, N=240, 2 chunks/bank). The f-gate is skipped entirely (c0=0) and
  gate columns are grouped [i|o|g] per chunk.
- gate PSUM is evacuated BY the activations: one strided Sigmoid over [i|o]
  and one Tanh over [g] read PSUM across banks directly (3-dim APs).
- h1 is written into persistent ping-pong tiles with a memset-once ones slot
  (6-slot blocks) -> L1 bias via ones row, same transpose/matmul scheme.
- final linear: t = h2 * W_lin (elementwise, weights tiled host-side),
  tensor_reduce over X, then Tanh with per-partition bias = b_lin.
- emission is split into 4 stages (A: L0 gates, B: tanh+h1, C: L1 gates,
  D: final) software-pipelined across supertiles; the x-load DMA is split
  per bank-group for earlier starts. xT staging uses one 2-bank PSUM tile
  (16 transposes, one DVE evacuation per supertile). An optional PAIR_LAG
  mode merging tc2(s)+tc1(s+lag) into one tanh measured net-negative
  (critical-path growth > instruction-overhead saving) and is off.
- nonzero h0/c0 (never produced by the spec) falls back to exact numpy.

Engine budget from the cost model (per core): ACT 57us busy / DVE 50 / PE 31
/ DMA 20; steady-state ACT occupancy ~93% - the kernel is bound by the
scalar engine's 41 transcendental elements/row plus per-instr overhead.
"""

import os
import numpy as np
import ml_dtypes

B = 1 << 20
IN_DIM, HID, OUT_DIM = 21, 5, 1
NCORES = 8
BC = B // NCORES          # 131072 rows per core
NBLK = BC // 128          # 1024 blocks per core
SUPERS = [64] * 16
NBMAX = max(SUPERS)
L0C = 4                   # L0 blocks per chunk
L1C = 16                  # L1 blocks per chunk
L0_PER_BANK = 8           # 8*60 = 480 <= 512 fp32
L1_PER_BANK = 2           # 2*240 = 480

_CACHE = {}
LAST_RESULTS = None


def _build_program(reps=1):
    import contextlib
    import concourse.bacc as bacc
    import concourse.tile as tile
    import concourse.mybir as mybir

    AF = mybir.ActivationFunctionType
    ALU = mybir.AluOpType
    BF16 = mybir.dt.bfloat16
    F32 = mybir.dt.float32
    nc = bacc.Bacc("TRN2", target_bir_lowering=False, debug=False, num_devices=NCORES)

    x_d = nc.declare_dram_parameter("xb", [BC, 22], BF16, isOutput=False)
    # all bf16 constants packed into one tensor -> one prologue DMA:
    # [ident 128 | w0blk 60 | w1blk 240 | wrep NBMAX*5]
    CW = 128 + L0C * 15 + L1C * 15 + NBMAX * HID
    cp_d = nc.declare_dram_parameter("cpack", [128, CW], BF16, isOutput=False)
    bl_d = nc.declare_dram_parameter("blin", [128, 1], F32, isOutput=False)
    y_d = nc.declare_dram_parameter("y", [BC, 1], F32, isOutput=True)

    env = lambda k, d: int(os.environ.get(k, d))
    with tile.TileContext(nc) as tc:
        with (
            tc.tile_pool(name="const", bufs=1) as constp,
            tc.tile_pool(name="xin", bufs=env("XIN_BUFS", 3)) as xinp,
            tc.tile_pool(name="xt_ps", bufs=env("XTPS_BUFS", 1), space="PSUM") as xtpsp,
            tc.tile_pool(name="xt_sb", bufs=env("XTSB_BUFS", 3)) as xtsbp,
            tc.tile_pool(name="g0_ps", bufs=env("G0_BUFS", 1), space="PSUM") as g0psp,
            tc.tile_pool(name="h1t_ps", bufs=env("H1TPS_BUFS", 1), space="PSUM") as h1tpsp,
            tc.tile_pool(name="h1t_sb", bufs=env("H1TSB_BUFS", 3)) as h1tsbp,
            tc.tile_pool(name="g1_ps", bufs=env("G1_BUFS", 1), space="PSUM") as g1psp,
            tc.tile_pool(name="acts", bufs=env("ACTS_BUFS", 3)) as actsp,
            tc.tile_pool(name="yout", bufs=env("YOUT_BUFS", 2)) as youtp,
        ):
            cp_sb = constp.tile([128, CW], BF16)
            nc.sync.dma_start(cp_sb[:], cp_d[:])
            id_sb = cp_sb[:, 0:128]
            w0_sb = cp_sb[0 : L0C * 22, 128 : 128 + L0C * 15]
            w1_sb = cp_sb[0 : L1C * 6, 128 + L0C * 15 : 128 + (L0C + L1C) * 15]
            wr_sb = cp_sb[:, 128 + (L0C + L1C) * 15 :]
            bl_sb = constp.tile([128, 1], F32)
            nc.gpsimd.dma_start(bl_sb[:], bl_d[:])
            # pre-trigger the sigmoid/tanh ACT table load so its ~2.7us
            # overlaps the first x-load/transpose/matmul instead of stalling
            # the first real gate activation
            warm = constp.tile([128, 2], BF16, tag="actwarm")
            nc.scalar.activation(warm[:, 0:1], id_sb[:, 0:1], AF.Sigmoid)
            nc.scalar.activation(warm[:, 1:2], id_sb[:, 0:1], AF.Tanh)

            # persistent ping-pong h1 tiles; ones slots memset once
            h1_tiles = []
            for pp in range(2):
                ht = constp.tile([128, NBMAX * 6], BF16, tag=f"h1tile{pp}")
                nc.vector.memset(
                    ht[:].rearrange("p (r f) -> p r f", f=6)[:, :, 5:6], 1.0
                )
                h1_tiles.append(ht)

            if reps > 1:
                rep_ctx = tc.For_i(0, reps, 1, hint_engines=tuple(nc.engines))
            else:
                rep_ctx = contextlib.nullcontext()
            QW = NBMAX * HID  # pair-half width (sized for largest supertile)

            def emit_A(si, nb, s0, cpair_prev):
                """x-load, L0 transposes+matmuls, sigmoid/tanh evac, c1-mul.
                c1(si) goes to cpair_prev[:, QW:2QW] (pair si-1) or standalone."""
                S = nb * 128
                n0ch = nb // L0C
                g0b = n0ch // L0_PER_BANK

                x_tile = xinp.tile([128, nb * 22], BF16, tag="xin")
                xs_v = x_d[s0 : s0 + S, :].rearrange("(p r) f -> p (r f)", p=128)
                grp = L0_PER_BANK * L0C * 22
                for bl in range(g0b):
                    nc.sync.dma_start(
                        out=x_tile[:, bl * grp : (bl + 1) * grp],
                        in_=xs_v[:, bl * grp : (bl + 1) * grp],
                    )

                g0_ps = g0psp.tile([128, g0b * 512], F32, tag="g0")
                nxt = env("XT_GRP", 16)
                for bl in range((n0ch + nxt - 1) // nxt):
                    ch_lo = bl * nxt
                    nch = min(nxt, n0ch - ch_lo)
                    xt_ps = xtpsp.tile([L0C * 22, nxt * 128], BF16, tag="xtps")
                    for c in range(nch):
                        nc.tensor.transpose(
                            xt_ps[:, c * 128 : (c + 1) * 128],
                            x_tile[:, (ch_lo + c) * L0C * 22 : (ch_lo + c + 1) * L0C * 22],
                            id_sb[:],
                        )
                    xt_sb = xtsbp.tile([L0C * 22, nxt * 128], BF16, tag="xtsb")
                    ncs = env("COPY_SPLIT", 1)
                    wtot = nch * 128
                    wh = (wtot // ncs + 127) // 128 * 128
                    for h in range(ncs):
                        lo, hi = h * wh, min((h + 1) * wh, wtot)
                        if lo < hi:
                            nc.vector.tensor_copy(xt_sb[:, lo:hi], xt_ps[:, lo:hi])
                    for c in range(nch):
                        cg = ch_lo + c
                        off = (cg // L0_PER_BANK) * 512 + (cg % L0_PER_BANK) * 60
                        nc.tensor.matmul(
                            g0_ps[:, off : off + 60],
                            xt_sb[:, c * 128 : (c + 1) * 128],
                            w0_sb[:],
                            start=True,
                            stop=True,
                        )

                g0v = (
                    g0_ps[:]
                    .rearrange("p (b x) -> p b x", x=512)[:, :, : L0_PER_BANK * 60]
                    .rearrange("p b (c n) -> p b c n", n=60)
                )
                sio0 = actsp.tile([128, n0ch * 40], BF16, tag="sio0")
                nc.scalar.activation(
                    sio0[:].rearrange("p (b c n) -> p b c n", n=40, c=L0_PER_BANK),
                    g0v[:, :, :, 0:40],
                    AF.Sigmoid,
                )
                tg0 = actsp.tile([128, nb * HID], BF16, tag="tg0")
                nc.scalar.activation(
                    tg0[:].rearrange("p (b c n) -> p b c n", n=20, c=L0_PER_BANK),
                    g0v[:, :, :, 40:60],
                    AF.Tanh,
                )
                sio0v = sio0[:].rearrange("p (c n) -> p c n", n=40)
                if cpair_prev is not None:
                    c1 = cpair_prev[:, QW : QW + nb * HID]
                else:
                    c1 = actsp.tile([128, nb * HID], BF16, tag="c1solo", name="c1solo")[:]
                nc.vector.tensor_mul(
                    c1.rearrange("p (c n) -> p c n", n=20),
                    sio0v[:, :, 0:20],
                    tg0[:].rearrange("p (c n) -> p c n", n=20),
                )
                return dict(si=si, nb=nb, s0=s0, sio0=sio0, c1solo=None if cpair_prev is not None else c1)

            def emit_B(ctxA, cpair_prev, ctx_prev):
                """pair-tanh (tc2(s-1) | tc1(s)) then h1-mul(s)."""
                si, nb, sio0 = ctxA["si"], ctxA["nb"], ctxA["sio0"]
                n0ch = nb // L0C
                if cpair_prev is not None:
                    tcpair = actsp.tile([128, 2 * QW], BF16, tag="tcpair")
                    nc.scalar.activation(tcpair[:], cpair_prev[:], AF.Tanh)
                    ctx_prev["tc2"] = tcpair[:, 0:QW]
                    tc1 = tcpair[:, QW : QW + nb * HID]
                else:
                    tc1t = actsp.tile([128, nb * HID], BF16, tag="tc1solo")
                    nc.scalar.activation(tc1t[:], ctxA["c1solo"], AF.Tanh)
                    tc1 = tc1t[:]
                h1 = h1_tiles[si % 2]
                nc.vector.tensor_mul(
                    h1[:].rearrange("p (c d f) -> p c d f", d=L0C, f=6)[:, :n0ch, :, 0:5],
                    sio0[:].rearrange("p (c g d f) -> p c g d f", g=2, d=L0C, f=5)[:, :, 1],
                    tc1.rearrange("p (c d f) -> p c d f", d=L0C, f=5),
                )
                ctxA["h1"] = h1
                return ctxA

            def emit_C(ctx):
                """L1 transposes+matmuls, sigmoid/tanh evac, c2-mul -> cpair[lo]."""
                si, nb, h1 = ctx["si"], ctx["nb"], ctx["h1"]
                n1ch = nb // L1C
                g1b = n1ch // L1_PER_BANK
                g1_ps = g1psp.tile([128, g1b * 512], F32, tag="g1")
                h1t_ps = h1tpsp.tile([L1C * 6, n1ch * 128], BF16, tag="h1tps")
                for c in range(n1ch):
                    nc.tensor.transpose(
                        h1t_ps[:, c * 128 : (c + 1) * 128],
                        h1[:, c * L1C * 6 : (c + 1) * L1C * 6],
                        id_sb[:],
                    )
                h1t_sb = h1tsbp.tile([L1C * 6, n1ch * 128], BF16, tag="h1tsb")
                nc.vector.tensor_copy(h1t_sb[:], h1t_ps[:])
                for c in range(n1ch):
                    off = (c // L1_PER_BANK) * 512 + (c % L1_PER_BANK) * 240
                    nc.tensor.matmul(
                        g1_ps[:, off : off + 240],
                        h1t_sb[:, c * 128 : (c + 1) * 128],
                        w1_sb[:],
                        start=True,
                        stop=True,
                    )

                g1v = (
                    g1_ps[:]
                    .rearrange("p (b x) -> p b x", x=512)[:, :, : L1_PER_BANK * 240]
                    .rearrange("p b (c n) -> p b c n", n=240)
                )
                sio1 = actsp.tile([128, n1ch * 160], BF16, tag="sio1")
                nc.scalar.activation(
                    sio1[:].rearrange("p (b c n) -> p b c n", n=160, c=L1_PER_BANK),
                    g1v[:, :, :, 0:160],
                    AF.Sigmoid,
                )
                tg1 = actsp.tile([128, nb * HID], BF16, tag="tg1")
                nc.scalar.activation(
                    tg1[:].rearrange("p (b c n) -> p b c n", n=80, c=L1_PER_BANK),
                    g1v[:, :, :, 160:240],
                    AF.Tanh,
                )
                cpair = actsp.tile([128, 2 * QW], BF16, tag="cpair", name="cpair")
                sio1v = sio1[:].rearrange("p (c n) -> p c n", n=160)
                nc.vector.tensor_mul(
                    cpair[:, 0 : nb * HID].rearrange("p (c n) -> p c n", n=80),
                    sio1v[:, :, 0:80],
                    tg1[:].rearrange("p (c n) -> p c n", n=80),
                )
                ctx["sio1"] = sio1
                ctx["cpair"] = cpair
                return ctx

            def emit_D(ctx):
                """vp, t, reduce, y-tanh, y-store (needs ctx["tc2"])."""
                si, nb, s0, sio1 = ctx["si"], ctx["nb"], ctx["s0"], ctx["sio1"]
                S = nb * 128
                sio1v = sio1[:].rearrange("p (c n) -> p c n", n=160)
                vp = actsp.tile([128, nb * HID], BF16, tag="vp")
                nc.vector.tensor_mul(
                    vp[:].rearrange("p (c n) -> p c n", n=80),
                    sio1v[:, :, 80:160],
                    ctx["tc2"][:, 0 : nb * HID].rearrange("p (c n) -> p c n", n=80),
                )
                t = actsp.tile([128, nb * HID], BF16, tag="t")
                nc.vector.tensor_mul(t[:], vp[:], wr_sb[:, : nb * HID])
                ypre = actsp.tile([128, nb], F32, tag="ypre")
                nc.vector.tensor_reduce(
                    ypre[:].rearrange("p (r o) -> p r o", o=1),
                    t[:].rearrange("p (r f) -> p r f", f=HID),
                    mybir.AxisListType.X,
                    ALU.add,
                )
                y_tile = youtp.tile([128, nb], F32, tag="y")
                nc.scalar.activation(y_tile[:], ypre[:], AF.Tanh, bias=bl_sb[:, 0:1])
                nc.sync.dma_start(
                    out=y_d[s0 : s0 + S, 0:1].rearrange("(p r) o -> p (r o)", p=128),
                    in_=y_tile[:],
                )

            with rep_ctx:
                nS = len(SUPERS)
                offs = [sum(SUPERS[:i]) * 128 for i in range(nS)]
                ctxs = [None] * nS
                LAG = env("PAIR_LAG", 0)  # 0 = no tanh pairing

                def solo_tc2(ctx):
                    w = ctx["nb"] * HID
                    tc2l = actsp.tile([128, QW], BF16, tag="tc2solo", name="tc2solo")
                    nc.scalar.activation(tc2l[:, 0:w], ctx["cpair"][:, 0:w], AF.Tanh)
                    ctx["tc2"] = tc2l[:]

                if LAG == 0:
                    prev = None
                    for s in range(nS):
                        a = emit_A(s, SUPERS[s], offs[s], None)
                        ctx = emit_B(a, None, None)
                        if prev is not None:
                            emit_C(prev)
                            solo_tc2(prev)
                            emit_D(prev)
                        prev = ctx
                    emit_C(prev)
                    solo_tc2(prev)
                    emit_D(prev)
                else:
                    for s in range(min(LAG, nS)):
                        a = emit_A(s, SUPERS[s], offs[s], None)
                        ctxs[s] = emit_B(a, None, None)
                        emit_C(ctxs[s])
                    for s in range(LAG, nS):
                        a = emit_A(s, SUPERS[s], offs[s], ctxs[s - LAG]["cpair"])
                        ctxs[s] = emit_B(a, ctxs[s - LAG]["cpair"], ctxs[s - LAG])
                        emit_D(ctxs[s - LAG])
                        ctxs[s - LAG] = None
                        emit_C(ctxs[s])
                    for s in range(max(nS - LAG, 0), nS):
                        solo_tc2(ctxs[s])
                        emit_D(ctxs[s])

    nc.compile()
    return nc


def _build_inputs(x, W_ih0, W_hh0, b_ih0, b_hh0, W_ih1, W_hh1, b_ih1, b_hh1, W_lin, b_lin):
    bf16 = ml_dtypes.bfloat16
    b0 = (np.asarray(b_ih0) + np.asarray(b_hh0)).astype(np.float32)
    b1 = (np.asarray(b_ih1) + np.asarray(b_hh1)).astype(np.float32)
    W0 = np.asarray(W_ih0, np.float32)
    W1 = np.asarray(W_ih1, np.float32)
    sel = {"i": range(0, 5), "g": range(10, 15), "o": range(15, 20)}
    gscale = {"i": 1.0, "o": 1.0, "g": 1.0}

    def blockdiag(W, b, chunk, slot, wscale):
        # rows: slot*dr + k  (k < kin: weights*gscale*wscale, k == kin: bias*gscale)
        kin = W.shape[1]
        out = np.zeros((chunk * slot, chunk * 15), np.float32)
        for dr in range(chunk):
            for grp, key in enumerate(("i", "o", "g")):
                gs = gscale[key]
                for kk, gr in enumerate(sel[key]):
                    col = grp * (chunk * 5) + dr * 5 + kk
                    r0 = dr * slot
                    out[r0 : r0 + kin, col] = W[gr, :] * gs * wscale
                    out[r0 + kin, col] = b[gr] * gs
        return out.astype(bf16)

    w0blk = blockdiag(W0, b0, L0C, 22, 1.0)
    w1blk = blockdiag(W1, b1, L1C, 6, 1.0)
    wrep = (
        np.tile(np.asarray(W_lin, np.float32)[0], NBMAX * 128)
        .reshape(128, NBMAX * HID)
        .astype(bf16)
    )
    blin = np.full((128, 1), float(np.asarray(b_lin)[0]), np.float32)
    ident = np.eye(128, dtype=bf16)
    CW = 128 + L0C * 15 + L1C * 15 + NBMAX * HID
    cpack = np.zeros((128, CW), bf16)
    cpack[:, 0:128] = ident
    cpack[0 : L0C * 22, 128 : 128 + L0C * 15] = w0blk
    cpack[0 : L1C * 6, 128 + L0C * 15 : 128 + (L0C + L1C) * 15] = w1blk
    cpack[:, 128 + (L0C + L1C) * 15 :] = wrep

    xb = np.empty((B, 22), bf16)
    xb[:, :21] = np.asarray(x, np.float32).astype(bf16)
    xb[:, 21] = bf16(1.0)

    in_maps = []
    for c in range(NCORES):
        in_maps.append(
            {
                "xb": xb[c * BC : (c + 1) * BC],
                "cpack": cpack,
                "blin": blin,
            }
        )
    return in_maps


def _reference_numpy(x, h0, c0, W_ih0, W_hh0, b_ih0, b_hh0, W_ih1, W_hh1, b_ih1, b_hh1, W_lin, b_lin):
    # general fallback (never taken for the spec'd zero-state inputs)
    def cell(x_, h, c, Wi, Wh, bi, bh):
        g = x_ @ Wi.T + h @ Wh.T + (bi + bh)
        i, f, gg, o = np.split(g, 4, axis=-1)
        sig = lambda z: 1.0 / (1.0 + np.exp(-z))
        cn = sig(f) * c + sig(i) * np.tanh(gg)
        return sig(o) * np.tanh(cn), cn

    h1, _ = cell(x, h0[0], c0[0], W_ih0, W_hh0, b_ih0, b_hh0)
    h2, _ = cell(h1, h0[1], c0[1], W_ih1, W_hh1, b_ih1, b_hh1)
    return np.tanh(h2 @ W_lin.T + b_lin).astype(np.float32)


def kernel(x, h0, c0, W_ih0, W_hh0, b_ih0, b_hh0, W_ih1, W_hh1, b_ih1, b_hh1, W_lin, b_lin):
    global LAST_RESULTS
    args = dict(
        x=np.asarray(x), h0=np.asarray(h0), c0=np.asarray(c0),
        W_ih0=np.asarray(W_ih0), W_hh0=np.asarray(W_hh0),
        b_ih0=np.asarray(b_ih0), b_hh0=np.asarray(b_hh0),
        W_ih1=np.asarray(W_ih1), W_hh1=np.asarray(W_hh1),
        b_ih1=np.asarray(b_ih1), b_hh1=np.asarray(b_hh1),
        W_lin=np.asarray(W_lin), b_lin=np.asarray(b_lin),
    )
    if np.any(args["h0"]) or np.any(args["c0"]):
        return _reference_numpy(**args)

    from concourse.bass_utils import run_bass_kernel_spmd

    if "nc" not in _CACHE:
        _CACHE["nc"] = _build_program()
    nc = _CACHE["nc"]

    in_maps = _build_inputs(
        args["x"], args["W_ih0"], args["W_hh0"], args["b_ih0"], args["b_hh0"],
        args["W_ih1"], args["W_hh1"], args["b_ih1"], args["b_hh1"],
        args["W_lin"], args["b_lin"],
    )
    trace = bool(int(os.environ.get("TRN_TRACE", "0")))
    res = run_bass_kernel_spmd(nc, in_maps, list(range(NCORES)), trace=trace)
    LAST_RESULTS = res
    return np.concatenate([res.results[i]["y"] for i in range(NCORES)], axis=0)
